# revision 34
# baseline (speedup 1.0000x reference)
"""TRN2 Bass kernel for nn_MultiHeadAttention_86878598464357.

reference:  qkv = x @ w_qkv.T (RoPE on q,k) -> causal softmax attention ->
            torch-faithful reshape [B,H,T,D]->[B,T,C] -> proj @ w_proj.T

Sharding (8 NeuronCores): tensor-parallel over heads, 2 heads per core.
Because the torch-faithful reshape makes output row t' depend only on head
t'//128, each core independently computes full output rows for its heads --
no collectives. Per core:
  - qkv projection for its 2 heads (bf16 matmuls, fp32 PSUM accumulation)
  - RoPE via elementwise ops against host-precomputed sin/cos tables
  - causal attention in transposed-score layout S^T[s,t] (no max-subtraction;
    scores are O(6) so exp is safe in fp32), denominator via ones-matmul,
    reciprocal computed in [128,x] layout, partition-broadcast, normalize
  - output projection with stride-16 lhsT access patterns implementing the
    reshape
Host side: transpose/cast inputs to bf16, build rope tables, scatter/gather.
"""
import math
from contextlib import ExitStack

import numpy as np

F32 = None
BF16 = None

B, T, C = 2, 2048, 2048
H, D = 16, 128
HL = 2
TB = 512
NTB = T // TB
NTT = T // 128
KT = C // 128
SCALE = 1.0 / math.sqrt(D)
N_CORES = 8

_CACHE = {}


def _interleave(gen_a, gen_b, ratio):
    a = list(gen_a)
    bs = list(gen_b)
    bi = 0
    for i, chunk in enumerate(a):
        chunk()
        take = int(round((i + 1) * ratio)) - int(round(i * ratio))
        for _ in range(take):
            if bi < len(bs):
                bs[bi]()
                bi += 1
    while bi < len(bs):
        bs[bi]()
        bi += 1


def _emit(nc, io, p, mybir):
    """Emit the full per-core forward pass as one merged pipeline.

    qkv tb-groups stream for both batches; attention work items are
    appended to a filler deque as soon as their tb-group dependencies
    complete and are drained between qkv chunks so the tensor queue always
    has DMA-independent work.  proj(b0) interleaves with leftover
    attention; proj(b1) runs last with double-queue weight prefetch.
    """
    from collections import deque

    F32 = mybir.dt.float32
    BF16 = mybir.dt.bfloat16

    w_re = io["w_qkv_bf"].rearrange("(kt p) f d -> p kt (f d)", p=128)
    w_kt = [
        p["const"].tile([128, 6 * 128], BF16, name=f"w_kt{kt}")
        for kt in range(KT)
    ]
    tri_sb = p["const"].tile([128, 128], BF16, name="tri_sb")
    sin_sb = p["const"].tile([64, T], BF16, name="sin_sb")
    cos_sb = p["const"].tile([64, T], BF16, name="cos_sb")
    ones_sb = p["const"].tile([128, 1], BF16, name="ones_sb")
    ones_row = p["const"].tile([1, 128], BF16, name="ones_row")

    def emit_consts(x_dmas):
        # Two HWDGE queues, ordered so w/x supply matches the warm chain's
        # kt-order consumption.  scalar: x(tb0) subtiles + odd w slices;
        # sync: even w slices + rope tables.
        nc.sync.dma_start(out=w_kt[0][:], in_=w_re[:, 0])
        nc.sync.dma_start(out=w_kt[2][:], in_=w_re[:, 2])
        scalar_seq = [x_dmas[0], 1, 3, x_dmas[1], 5, 7,
                      x_dmas[2], 9, 11, x_dmas[3], 13, 15]
        for it in scalar_seq:
            if callable(it):
                it()
            else:
                nc.scalar.dma_start(out=w_kt[it][:], in_=w_re[:, it])
        nc.sync.dma_start(out=sin_sb[:], in_=io["sin_t"][:])
        nc.sync.dma_start(out=cos_sb[:], in_=io["cos_t"][:])
        nc.sync.dma_start(out=tri_sb[:], in_=io["tri"][:])
        nc.vector.memset(ones_sb[:], 1.0)
        nc.vector.memset(ones_row[:], 1.0)
        for kt in range(4, KT, 2):
            nc.sync.dma_start(out=w_kt[kt][:], in_=w_re[:, kt])

    def wslice(kt, fb):
        return w_kt[kt][:, fb * 128 : (fb + 1) * 128]

    qkv_t = {}
    out_sb = {}
    rope_q = deque()

    def emit_rope(psum, dst, h, ts):
        t1 = p["misc"].tile([64, TB], F32, name="rope_t1")
        t2 = p["misc"].tile([64, TB], F32, name="rope_t2")
        nc.vector.tensor_mul(t1[:], psum[0:64, :], cos_sb[:, ts])
        nc.vector.tensor_mul(t2[:], psum[64:128, :], sin_sb[:, ts])
        nc.vector.tensor_sub(dst[h][0:64, ts], t1[:], t2[:])
        nc.vector.tensor_mul(t1[:], psum[64:128, :], cos_sb[:, ts])
        nc.vector.tensor_mul(t2[:], psum[0:64, :], sin_sb[:, ts])
        nc.vector.tensor_add(dst[h][64:128, ts], t1[:], t2[:])

    def make_qkv(b):
        q = {h: p["qkvp"].tile([128, T], BF16, name=f"q_sb_{h}") for h in range(HL)}
        k = {h: p["qkvp"].tile([128, T], BF16, name=f"k_sb_{h}") for h in range(HL)}
        v = p["qkvp"].tile([128, NTT, HL * 128], BF16, name="v_sb")
        qkv_t[b] = (q, k, v)
        x_holder = {}
        fbmap = [(q, 0), (q, 1), (k, 0), (k, 1)]

        def load_x(tb):
            def f():
                xs = [
                    p["xp"].tile([128, 4, TB], BF16, name=f"x_sb{i}")
                    for i in range(4)
                ]
                x_holder[tb] = xs

                def one(i, eng):
                    def g():
                        eng.dma_start(
                            out=xs[i][:],
                            in_=io["x_bf"][b][
                                i * 512 : (i + 1) * 512,
                                tb * TB : (tb + 1) * TB,
                            ].rearrange("(k p) t -> p k t", p=128),
                        )
                    return g

                if b == 0 and tb == 0:
                    emit_consts([one(i, nc.scalar) for i in range(4)])
                else:
                    for i in range(4):
                        one(i, nc.sync)()
            return f

        def qk_warm():
            # tb0 of b0: interleave the four qk chains at kt4 granularity so
            # consumption matches DMA supply order (each w slice used 4x
            # before the next is needed).  Uses ps_mm + ps_s banks (attention
            # has not started yet).
            def f():
                xs = x_holder[0]
                ts = slice(0, TB)
                psums = [
                    p["ps_mm"].tile([128, TB], F32, name="qk_psum", tag="mmps"),
                    p["ps_mm"].tile([128, TB], F32, name="qk_psum", tag="mmps"),
                    p["ps_s"].tile([128, TB], F32, name="s_ps", tag="sps"),
                    p["ps_s"].tile([128, TB], F32, name="s_ps", tag="sps"),
                ]
                for k4 in range(4):
                    for fb in range(4):
                        for kt in range(4 * k4, 4 * k4 + 4):
                            nc.tensor.matmul(
                                psums[fb][:],
                                wslice(kt, fb),
                                rhs=xs[kt // 4][:, kt % 4],
                                start=(kt == 0),
                                stop=(kt == KT - 1),
                            )
                for fb, (dst, h) in enumerate(fbmap):
                    emit_rope(psums[fb], dst, h, ts)
            return f

        def qk_chunk(tb, fb):
            def f():
                xs = x_holder[tb]
                dst, h = fbmap[fb]
                ts = slice(tb * TB, (tb + 1) * TB)
                psum = p["ps_mm"].tile([128, TB], F32, name="qk_psum", tag="mmps")
                for kt in range(KT):
                    nc.tensor.matmul(
                        psum[:],
                        wslice(kt, fb),
                        rhs=xs[kt // 4][:, kt % 4],
                        start=(kt == 0),
                        stop=(kt == KT - 1),
                    )
                emit_rope(psum, dst, h, ts)
            return f

        def v_chunk(tb, tl):
            def f():
                xs = x_holder[tb]
                tt = tb * 4 + tl
                psum = p["ps_mm"].tile([128, HL * 128], F32, name="v_psum",
                                       tag="mmps")
                for kt in range(KT):
                    nc.tensor.matmul(
                        psum[:],
                        xs[kt // 4][:, kt % 4, tl * 128 : (tl + 1) * 128],
                        rhs=w_kt[kt][:, 4 * 128 : 6 * 128],
                        start=(kt == 0),
                        stop=(kt == KT - 1),
                    )
                nc.scalar.copy(v[:, tt], psum[:])
            return f

        groups = []
        for tb in range(NTB):
            g = [load_x(tb)]
            if b == 0 and tb == 0:
                g.append(qk_warm())
            else:
                g += [qk_chunk(tb, fb) for fb in range(4)]
            g += [v_chunk(tb, tl) for tl in range(4)]
            groups.append(g)
        return groups

    def attn_block_items(b, h, tb):
        if (b, h) not in out_sb:
            out_sb[(b, h)] = p["outp"].tile([128, T], BF16, name=f"o_sb_{b}_{h}")
        o_sb = out_sb[(b, h)]
        njs = tb * 4 + 4
        e_acc = p["accp"].tile([128, TB], BF16, name="e_acc")
        o_ps = p["ps_o"].tile([128, TB], F32, name="o_ps", tag="ops")

        e_tiles = {}

        def sf(j):
            q, k, v = qkv_t[b]
            c0 = max(0, j * 128 - tb * TB)
            cs = slice(c0, TB)
            tcs = slice(tb * TB + c0, (tb + 1) * TB)
            s_ps = p["ps_s"].tile([128, TB], F32, name="s_ps", tag="sps")
            nc.tensor.matmul(
                s_ps[:, cs],
                k[h][:, j * 128 : (j + 1) * 128],
                rhs=q[h][:, tcs],
                start=True,
                stop=True,
            )
            e_sb = p["ep"].tile([128, TB], BF16, name="e_sb", tag="e")
            nc.scalar.activation(
                e_sb[:, cs],
                s_ps[:, cs],
                mybir.ActivationFunctionType.Exp,
                scale=SCALE,
            )
            if j >= tb * 4:
                dcs = slice(c0, c0 + 128)
                nc.vector.tensor_mul(e_sb[:, dcs], e_sb[:, dcs], tri_sb[:])
            e_tiles[j] = e_sb

        def of(j):
            q, k, v = qkv_t[b]
            c0 = max(0, j * 128 - tb * TB)
            cs = slice(c0, TB)
            e_sb = e_tiles.pop(j)
            nc.tensor.matmul(
                o_ps[:, cs],
                v[:, j, h * 128 : (h + 1) * 128],
                rhs=e_sb[:, cs],
                start=(j == 0),
                stop=(j == njs - 1),
            )
            if j == 0:
                nc.vector.tensor_copy(e_acc[:], e_sb[:])
            else:
                nc.vector.tensor_add(e_acc[:, cs], e_acc[:, cs], e_sb[:, cs])

        def fin():
            ts = slice(tb * TB, (tb + 1) * TB)
            d_ps = p["ps_d"].tile([1, TB], F32, name="d_ps", tag="dps")
            nc.tensor.matmul(
                d_ps[:], ones_sb[:], rhs=e_acc[:], start=True, stop=True
            )
            den_sb = p["misc"].tile([1, TB], F32, name="den_sb")
            nc.vector.tensor_copy(den_sb[:], d_ps[:])
            den128 = p["misc"].tile([128, TB // 128], F32, name="den128")
            nc.sync.dma_start(out=den128[:], in_=den_sb[0:1, :])
            rec128 = p["misc"].tile([128, TB // 128], BF16, name="rec128")
            with nc.allow_low_precision(reason="1/d broadcast via bf16 matmul"):
                nc.vector.reciprocal(rec128[:], den128[:])
            rec_sb = p["misc"].tile([1, TB], BF16, name="rec_sb")
            nc.sync.dma_start(out=rec_sb[0:1, :], in_=rec128[:])
            # broadcast 1/d across partitions with a rank-1 ones matmul
            # (gpsimd partition_broadcast drains a slow software-DMA queue)
            rb_ps = p["ps_d"].tile([128, TB], F32, name="rb_ps", tag="dps")
            nc.tensor.matmul(
                rb_ps[:], ones_row[:], rhs=rec_sb[:], start=True, stop=True
            )
            rb_sb = p["misc"].tile([128, TB], F32, name="rb_sb")
            nc.scalar.copy(rb_sb[:], rb_ps[:])
            nc.vector.tensor_mul(o_sb[:, ts], o_ps[:], rb_sb[:])

        # software pipeline: S(j+1) emitted before o(j) so the exp latency
        # of e(j) hides under S(j+1)'s matmul + interleaved filler
        items = [lambda: sf(0)]
        for j in range(1, njs):
            items.append(lambda j=j: (sf(j), of(j - 1)) and None)
        items.append(lambda: of(njs - 1))
        items.append(fin)
        return items

    TBP = 256  # proj output-column chunk (8 chunks; small wt tiles)

    def make_proj(b):
        wt_holder = {}

        def load_wt(ob):
            def f():
                wt = p["wp"].tile([128, KT, TBP], BF16, name="wt")
                # host pre-packed [ob, p, kt, o]: contiguous 8KB/partition
                nc.sync.dma_start(out=wt[:], in_=io["w_proj_bf"][ob])
                wt_holder[ob] = wt
            return f

        def pchunk(ob, h):
            def f():
                wt = wt_holder[ob]
                os_ = slice(ob * TBP, (ob + 1) * TBP)
                y_ps = p["ps_mm"].tile([128, TBP], F32, name="y_ps", tag="mmps")
                o_sb = out_sb[(b, h)]
                for kt in range(KT):
                    lhsT = o_sb.rearrange("d (t2 g) -> d g t2", g=16)[:, kt]
                    nc.tensor.matmul(
                        y_ps[:],
                        lhsT,
                        rhs=wt[:, kt],
                        start=(kt == 0),
                        stop=(kt == KT - 1),
                    )
                y_sb = p["misc"].tile([128, TBP], F32, name="y_sb")
                nc.scalar.copy(y_sb[:], y_ps[:])
                # b=1 stores go out on the scalar queue: the sync queue is
                # saturated with wt reloads during the proj(b1) phase
                eng = nc.scalar if b == 1 else nc.sync
                eng.dma_start(out=io["y"][b, h, :, os_], in_=y_sb[:])
            return f

        return load_wt, pchunk

    # ---- merged pipeline ----
    filler = deque()

    def drain(n):
        for _ in range(n):
            if not filler:
                return
            filler.popleft()()

    for b in (0, 1):
        groups = make_qkv(b)
        for tb in range(NTB):
            for c in groups[tb]:
                c()
                drain(2)
            for h in range(HL):
                filler.extend(attn_block_items(b, h, tb))

    lw0, pc0 = make_proj(0)
    lw1, pc1 = make_proj(1)
    NOB = C // TBP
    c_items = []
    for ob in range(NOB - 1):
        c_items += [lw0(ob), pc0(ob, 0), pc0(ob, 1)]
    # prefetch first two b=1 wt chunks during phase C (wp bufs=4)
    c_items += [lw0(NOB - 1), lw1(0), lw1(1), pc0(NOB - 1, 0), lw1(2),
                pc0(NOB - 1, 1), lw1(3), lw1(4), lw1(5)]
    rem = list(filler)
    filler.clear()
    # Interleave leftover attention only with the first ~60% of proj(b0):
    # the proj-only tail covers the last finalize chains' latency so
    # proj(b1) never waits on a just-normalized o_sb.
    nhead = max(1, (len(c_items) * 3) // 5)
    _interleave(iter(c_items[:nhead]), iter(rem),
                ratio=max(1, len(rem)) / nhead)
    for f in c_items[nhead:]:
        f()
    d_items = []
    for ob in range(NOB):
        d_items += [pc1(ob, 0), pc1(ob, 1)]
        if ob + 6 < NOB:
            # reload into the buffer pc1(ob,*) just released (wp bufs=6)
            d_items.append(lw1(ob + 6))
    for f in d_items:
        f()


def _build(reps=1):
    from concourse import bacc
    import concourse.mybir as mybir
    import concourse.tile as tile

    F32 = mybir.dt.float32
    BF16 = mybir.dt.bfloat16

    nc = bacc.Bacc(None, target_bir_lowering=False)
    io = {
        "x_bf": nc.dram_tensor("x_bf", [B, C, T], BF16, kind="ExternalInput"),
        "w_qkv_bf": nc.dram_tensor("w_qkv_bf", [C, 6, 128], BF16,
                                   kind="ExternalInput"),
        "w_proj_bf": nc.dram_tensor("w_proj_bf", [C // 256, 128, C // 128, 256],
                                    BF16, kind="ExternalInput"),
        "sin_t": nc.dram_tensor("sin_t", [64, T], BF16, kind="ExternalInput"),
        "cos_t": nc.dram_tensor("cos_t", [64, T], BF16, kind="ExternalInput"),
        "tri": nc.dram_tensor("tri", [128, 128], BF16, kind="ExternalInput"),
        "y": nc.dram_tensor("y", [B, HL, 128, C], F32, kind="ExternalOutput"),
    }
    with tile.TileContext(nc) as tc, ExitStack() as ctx:
        pools = {
            "const": ctx.enter_context(tc.tile_pool(name="const", bufs=1)),
            "ps_mm": ctx.enter_context(
                tc.tile_pool(name="ps_mm", bufs=2, space="PSUM")),
            "ps_s": ctx.enter_context(
                tc.tile_pool(name="ps_s", bufs=2, space="PSUM")),
            "ps_o": ctx.enter_context(
                tc.tile_pool(name="ps_o", bufs=2, space="PSUM")),
            "ps_d": ctx.enter_context(
                tc.tile_pool(name="ps_d", bufs=2, space="PSUM")),
            "xp": ctx.enter_context(tc.tile_pool(name="xp", bufs=2)),
            "qkvp": ctx.enter_context(tc.tile_pool(name="qkvp", bufs=2)),
            "ep": ctx.enter_context(tc.tile_pool(name="ep", bufs=4)),
            "accp": ctx.enter_context(tc.tile_pool(name="accp", bufs=2)),
            "outp": ctx.enter_context(tc.tile_pool(name="outp", bufs=1)),
            "wp": ctx.enter_context(tc.tile_pool(name="wp", bufs=6)),
            "misc": ctx.enter_context(tc.tile_pool(name="misc", bufs=2)),
        }
        for _ in range(reps):
            _emit(nc, io, pools, mybir)
    nc.compile()
    return nc


def _make_executor(nc):
    import jax
    from jax.sharding import Mesh, NamedSharding, PartitionSpec
    from jax.experimental.shard_map import shard_map
    import concourse.mybir as mybir
    from concourse.bass2jax import (
        _bass_exec_p,
        install_neuronx_cc_hook,
        partition_id_tensor,
    )

    install_neuronx_cc_hook()
    partition_name = (
        nc.partition_id_tensor.name if nc.partition_id_tensor else None
    )
    in_names, out_names, out_avals, zero_outs = [], [], [], []
    for alloc in nc.m.functions[0].allocations:
        if not isinstance(alloc, mybir.MemoryLocationSet):
            continue
        name = alloc.memorylocations[0].name
        if alloc.kind == "ExternalInput":
            if name != partition_name:
                in_names.append(name)
        elif alloc.kind == "ExternalOutput":
            shape = tuple(alloc.tensor_shape)
            dtype = mybir.dt.np(alloc.dtype)
            out_names.append(name)
            out_avals.append(jax.core.ShapedArray(shape, dtype))
            zero_outs.append(np.zeros(shape, dtype))
    n_params = len(in_names)
    n_outs = len(out_avals)
    in_names.extend(out_names)
    if partition_name is not None:
        in_names.append(partition_name)
    donate = tuple(range(n_params, n_params + n_outs))

    def _body(*args):
        operands = list(args)
        if partition_name is not None:
            operands.append(partition_id_tensor())
        return tuple(
            _bass_exec_p.bind(
                *operands,
                out_avals=tuple(out_avals),
                in_names=tuple(in_names),
                out_names=tuple(out_names),
                lowering_input_output_aliases=(),
                sim_require_finite=True,
                sim_require_nnan=True,
                nc=nc,
            )
        )

    devices = jax.devices()[:N_CORES]
    assert len(devices) == N_CORES, f"need {N_CORES} cores, got {len(devices)}"
    mesh = Mesh(np.asarray(devices), ("core",))
    in_specs = (PartitionSpec("core"),) * (n_params + n_outs)
    out_specs = (PartitionSpec("core"),) * len(out_names)
    sharded = jax.jit(
        shard_map(_body, mesh=mesh, in_specs=in_specs, out_specs=out_specs,
                  check_rep=False),
        donate_argnums=donate,
        keep_unused=True,
    )

    def run(in_maps):
        per_core = [
            [np.asarray(m[name]) for name in in_names[:n_params]]
            for m in in_maps
        ]
        concat_in = [
            np.concatenate([per_core[c][i] for c in range(N_CORES)], axis=0)
            for i in range(n_params)
        ]
        concat_zeros = [
            np.zeros((N_CORES * z.shape[0], *z.shape[1:]), z.dtype)
            for z in zero_outs
        ]
        out_arrs = sharded(*concat_in, *concat_zeros)
        jax.block_until_ready(out_arrs)
        return [
            {
                name: np.asarray(out_arrs[i]).reshape(
                    N_CORES, *out_avals[i].shape
                )[c]
                for i, name in enumerate(out_names)
            }
            for c in range(N_CORES)
        ]

    return run


def _host_prep(x, w_qkv, w_proj):
    import ml_dtypes

    bf = ml_dtypes.bfloat16
    x = np.asarray(x, dtype=np.float32)
    w_qkv = np.asarray(w_qkv, dtype=np.float32)
    w_proj = np.asarray(w_proj, dtype=np.float32)

    x_bf = np.ascontiguousarray(x.transpose(0, 2, 1)).astype(bf)  # [B, C, T]
    # [c_in, o] -> [ob, p, kt, o_local]: the proj wt tile layout, so each
    # chunk DMA reads contiguous 8KB per partition
    wp_t = w_proj.T.reshape(16, 128, 8, 256)
    w_proj_bf = np.ascontiguousarray(wp_t.transpose(2, 1, 0, 3)).astype(bf)

    pos = np.arange(T, dtype=np.float32)[:, None]
    inv = np.exp(
        np.arange(0, D, 2, dtype=np.float32) * np.float32(-math.log(10000.0) / D)
    )
    ang = pos * inv
    sin_t = np.ascontiguousarray(np.sin(ang).astype(np.float32).T).astype(bf)
    cos_t = np.ascontiguousarray(np.cos(ang).astype(np.float32).T).astype(bf)
    tri = np.triu(np.ones((128, 128), dtype=np.float32)).astype(bf)

    in_maps = []
    for c in range(N_CORES):
        h0, h1 = 2 * c, 2 * c + 1
        blocks = []
        for base in (0, C, 2 * C):  # q, k, v feature rows
            for h in (h0, h1):
                blocks.append(w_qkv[base + h * D : base + (h + 1) * D, :])
        w_slab = np.stack(blocks, 0)  # [6, 128, C]
        w_t = np.ascontiguousarray(w_slab.transpose(2, 0, 1)).astype(bf)
        in_maps.append(
            {
                "x_bf": x_bf,
                "w_qkv_bf": w_t,
                "w_proj_bf": w_proj_bf,
                "sin_t": sin_t,
                "cos_t": cos_t,
                "tri": tri,
            }
        )
    return in_maps


def kernel(x, w_qkv, w_proj):
    """Full inputs in, full output out. Shards over 8 NeuronCores inside."""
    if "run" not in _CACHE:
        nc = _build()
        _CACHE["run"] = _make_executor(nc)
    run = _CACHE["run"]
    in_maps = _host_prep(x, w_qkv, w_proj)
    outs = run(in_maps)
    y = np.empty((B, T, C), dtype=np.float32)
    for c in range(N_CORES):
        for hl in range(HL):
            h = 2 * c + hl
            y[:, h * 128 : (h + 1) * 128, :] = outs[c]["y"][:, hl]
    return y



# revision 37
# speedup vs baseline: 1.0034x; 1.0034x over previous
"""TRN2 Bass kernel for nn_MultiHeadAttention_86878598464357.

reference:  qkv = x @ w_qkv.T (RoPE on q,k) -> causal softmax attention ->
            torch-faithful reshape [B,H,T,D]->[B,T,C] -> proj @ w_proj.T

Sharding (8 NeuronCores): tensor-parallel over heads, 2 heads per core.
Because the torch-faithful reshape makes output row t' depend only on head
t'//128, each core independently computes full output rows for its heads --
no collectives. Per core, one merged pipeline:
  - qkv projection (bf16 matmuls, fp32 PSUM), first tb-group supply-matched
    to the two HWDGE queues' DMA arrival order
  - RoPE on the vector engine against host-precomputed bf16 sin/cos tables
  - causal attention in transposed-score layout S^T[s,t] (no max-subtraction;
    scores are O(6) so fp32 exp is safe), software-pipelined S(j+1)-before-
    o(j); attention items drain between qkv chunks as filler so exp latency
    hides under independent matmuls
  - softmax denominator: masked exp tiles accumulated on the vector engine,
    one ones-matmul per (b,h,tb); 1/d broadcast across partitions via a
    rank-1 ones-row matmul (gpsimd partition_broadcast stalls on SWDGE)
  - output projection with stride-16 lhsT access patterns implementing the
    reshape; w_proj host-packed per 256-col chunk for contiguous DMA, deep
    prefetch (wp bufs=6) covers the b=1 reload
Host side: transpose/cast inputs to bf16, build rope tables, scatter/gather.
"""
import math
from contextlib import ExitStack

import numpy as np

F32 = None
BF16 = None

B, T, C = 2, 2048, 2048
H, D = 16, 128
HL = 2
TB = 512
NTB = T // TB
NTT = T // 128
KT = C // 128
SCALE = 1.0 / math.sqrt(D)
N_CORES = 8

_CACHE = {}


def _interleave(gen_a, gen_b, ratio):
    a = list(gen_a)
    bs = list(gen_b)
    bi = 0
    for i, chunk in enumerate(a):
        chunk()
        take = int(round((i + 1) * ratio)) - int(round(i * ratio))
        for _ in range(take):
            if bi < len(bs):
                bs[bi]()
                bi += 1
    while bi < len(bs):
        bs[bi]()
        bi += 1


def _emit(nc, io, p, mybir):
    """Emit the full per-core forward pass as one merged pipeline.

    qkv tb-groups stream for both batches; attention work items are
    appended to a filler deque as soon as their tb-group dependencies
    complete and are drained between qkv chunks so the tensor queue always
    has DMA-independent work.  proj(b0) interleaves with leftover
    attention; proj(b1) runs last with double-queue weight prefetch.
    """
    from collections import deque

    F32 = mybir.dt.float32
    BF16 = mybir.dt.bfloat16

    w_re = io["w_qkv_bf"].rearrange("(k4 k p) f d -> p k4 k (f d)", p=128, k=4)
    w4 = [
        p["const"].tile([128, 4, 6 * 128], BF16, name=f"w4_{k4}")
        for k4 in range(4)
    ]
    tri_sb = p["const"].tile([128, 128], BF16, name="tri_sb")
    sin_sb = p["const"].tile([64, T], BF16, name="sin_sb")
    cos_sb = p["const"].tile([64, T], BF16, name="cos_sb")
    ones_sb = p["const"].tile([128, 1], BF16, name="ones_sb")
    ones_row = p["const"].tile([1, 128], BF16, name="ones_row")

    def emit_consts(x_dmas):
        # Two HWDGE queues, ordered so w/x supply matches the warm chain's
        # k4-round consumption.  sync: w rounds 0,2 + rope tables;
        # scalar: x(tb0) subtiles + w rounds 1,3.
        nc.sync.dma_start(out=w4[0][:], in_=w_re[:, 0])
        x_dmas[0]()
        nc.scalar.dma_start(out=w4[1][:], in_=w_re[:, 1])
        nc.sync.dma_start(out=w4[2][:], in_=w_re[:, 2])
        x_dmas[1]()
        x_dmas[2]()
        nc.scalar.dma_start(out=w4[3][:], in_=w_re[:, 3])
        x_dmas[3]()
        nc.sync.dma_start(out=sin_sb[:], in_=io["sin_t"][:])
        nc.sync.dma_start(out=cos_sb[:], in_=io["cos_t"][:])
        nc.sync.dma_start(out=tri_sb[:], in_=io["tri"][:])
        nc.vector.memset(ones_sb[:], 1.0)
        nc.vector.memset(ones_row[:], 1.0)

    def wslice(kt, fb):
        return w4[kt // 4][:, kt % 4, fb * 128 : (fb + 1) * 128]

    qkv_t = {}
    out_sb = {}

    def emit_rope(psum, dst, h, ts):
        t1 = p["misc"].tile([64, TB], F32, name="rope_t1")
        t2 = p["misc"].tile([64, TB], F32, name="rope_t2")
        nc.vector.tensor_mul(t1[:], psum[0:64, :], cos_sb[:, ts])
        nc.vector.tensor_mul(t2[:], psum[64:128, :], sin_sb[:, ts])
        nc.vector.tensor_sub(dst[h][0:64, ts], t1[:], t2[:])
        nc.vector.tensor_mul(t1[:], psum[64:128, :], cos_sb[:, ts])
        nc.vector.tensor_mul(t2[:], psum[0:64, :], sin_sb[:, ts])
        nc.vector.tensor_add(dst[h][64:128, ts], t1[:], t2[:])

    def make_qkv(b):
        q = {h: p["qkvp"].tile([128, T], BF16, name=f"q_sb_{h}") for h in range(HL)}
        k = {h: p["qkvp"].tile([128, T], BF16, name=f"k_sb_{h}") for h in range(HL)}
        v = p["qkvp"].tile([128, NTT, HL * 128], BF16, name="v_sb")
        qkv_t[b] = (q, k, v)
        x_holder = {}
        fbmap = [(q, 0), (q, 1), (k, 0), (k, 1)]

        def load_x(tb):
            def f():
                xs = [
                    p["xp"].tile([128, 4, TB], BF16, name=f"x_sb{i}")
                    for i in range(4)
                ]
                x_holder[tb] = xs

                def one(i, eng):
                    def g():
                        eng.dma_start(
                            out=xs[i][:],
                            in_=io["x_bf"][b][
                                i * 512 : (i + 1) * 512,
                                tb * TB : (tb + 1) * TB,
                            ].rearrange("(k p) t -> p k t", p=128),
                        )
                    return g

                if b == 0 and tb == 0:
                    emit_consts([one(i, nc.scalar) for i in range(4)])
                else:
                    for i in range(4):
                        one(i, nc.sync)()
            return f

        def qk_warm():
            # tb0 of b0: interleave the four qk chains at kt4 granularity so
            # consumption matches DMA supply order (each w slice used 4x
            # before the next is needed).  Uses ps_mm + ps_s banks (attention
            # has not started yet).
            def f():
                xs = x_holder[0]
                ts = slice(0, TB)
                psums = [
                    p["ps_mm"].tile([128, TB], F32, name="qk_psum", tag="mmps"),
                    p["ps_mm"].tile([128, TB], F32, name="qk_psum", tag="mmps"),
                    p["ps_s"].tile([128, TB], F32, name="s_ps", tag="sps"),
                    p["ps_s"].tile([128, TB], F32, name="s_ps", tag="sps"),
                ]
                for k4 in range(4):
                    for fb in range(4):
                        for kt in range(4 * k4, 4 * k4 + 4):
                            nc.tensor.matmul(
                                psums[fb][:],
                                wslice(kt, fb),
                                rhs=xs[kt // 4][:, kt % 4],
                                start=(kt == 0),
                                stop=(kt == KT - 1),
                            )
                for fb, (dst, h) in enumerate(fbmap):
                    emit_rope(psums[fb], dst, h, ts)
            return f

        def qk_chunk(tb, fb):
            def f():
                xs = x_holder[tb]
                dst, h = fbmap[fb]
                ts = slice(tb * TB, (tb + 1) * TB)
                psum = p["ps_mm"].tile([128, TB], F32, name="qk_psum", tag="mmps")
                for kt in range(KT):
                    nc.tensor.matmul(
                        psum[:],
                        wslice(kt, fb),
                        rhs=xs[kt // 4][:, kt % 4],
                        start=(kt == 0),
                        stop=(kt == KT - 1),
                    )
                emit_rope(psum, dst, h, ts)
            return f

        def v_chunk(tb, tl):
            def f():
                xs = x_holder[tb]
                tt = tb * 4 + tl
                psum = p["ps_mm"].tile([128, HL * 128], F32, name="v_psum",
                                       tag="mmps")
                for kt in range(KT):
                    nc.tensor.matmul(
                        psum[:],
                        xs[kt // 4][:, kt % 4, tl * 128 : (tl + 1) * 128],
                        rhs=w4[kt // 4][:, kt % 4, 4 * 128 : 6 * 128],
                        start=(kt == 0),
                        stop=(kt == KT - 1),
                    )
                nc.scalar.copy(v[:, tt], psum[:])
            return f

        groups = []
        for tb in range(NTB):
            g = [load_x(tb)]
            if b == 0 and tb == 0:
                g.append(qk_warm())
            else:
                g += [qk_chunk(tb, fb) for fb in range(4)]
            g += [v_chunk(tb, tl) for tl in range(4)]
            groups.append(g)
        return groups

    def attn_block_items(b, h, tb):
        if (b, h) not in out_sb:
            out_sb[(b, h)] = p["outp"].tile([128, T], BF16, name=f"o_sb_{b}_{h}")
        o_sb = out_sb[(b, h)]
        njs = tb * 4 + 4
        e_acc = p["accp"].tile([128, TB], BF16, name="e_acc")
        o_ps = p["ps_o"].tile([128, TB], F32, name="o_ps", tag="ops")

        e_tiles = {}

        def sf(j):
            q, k, v = qkv_t[b]
            c0 = max(0, j * 128 - tb * TB)
            cs = slice(c0, TB)
            tcs = slice(tb * TB + c0, (tb + 1) * TB)
            s_ps = p["ps_s"].tile([128, TB], F32, name="s_ps", tag="sps")
            nc.tensor.matmul(
                s_ps[:, cs],
                k[h][:, j * 128 : (j + 1) * 128],
                rhs=q[h][:, tcs],
                start=True,
                stop=True,
            )
            e_sb = p["ep"].tile([128, TB], BF16, name="e_sb", tag="e")
            nc.scalar.activation(
                e_sb[:, cs],
                s_ps[:, cs],
                mybir.ActivationFunctionType.Exp,
                scale=SCALE,
            )
            if j >= tb * 4:
                dcs = slice(c0, c0 + 128)
                nc.vector.tensor_mul(e_sb[:, dcs], e_sb[:, dcs], tri_sb[:])
            e_tiles[j] = e_sb

        def of(j):
            q, k, v = qkv_t[b]
            c0 = max(0, j * 128 - tb * TB)
            cs = slice(c0, TB)
            e_sb = e_tiles.pop(j)
            nc.tensor.matmul(
                o_ps[:, cs],
                v[:, j, h * 128 : (h + 1) * 128],
                rhs=e_sb[:, cs],
                start=(j == 0),
                stop=(j == njs - 1),
            )
            if j == 0:
                nc.vector.tensor_copy(e_acc[:], e_sb[:])
            else:
                nc.vector.tensor_add(e_acc[:, cs], e_acc[:, cs], e_sb[:, cs])

        def fin():
            ts = slice(tb * TB, (tb + 1) * TB)
            d_ps = p["ps_d"].tile([1, TB], F32, name="d_ps", tag="dps")
            nc.tensor.matmul(
                d_ps[:], ones_sb[:], rhs=e_acc[:], start=True, stop=True
            )
            den_sb = p["misc"].tile([1, TB], F32, name="den_sb")
            nc.vector.tensor_copy(den_sb[:], d_ps[:])
            den128 = p["misc"].tile([128, TB // 128], F32, name="den128")
            nc.sync.dma_start(out=den128[:], in_=den_sb[0:1, :])
            rec128 = p["misc"].tile([128, TB // 128], BF16, name="rec128")
            with nc.allow_low_precision(reason="1/d broadcast via bf16 matmul"):
                nc.vector.reciprocal(rec128[:], den128[:])
            rec_sb = p["misc"].tile([1, TB], BF16, name="rec_sb")
            nc.sync.dma_start(out=rec_sb[0:1, :], in_=rec128[:])
            # broadcast 1/d across partitions with a rank-1 ones matmul
            # (gpsimd partition_broadcast drains a slow software-DMA queue)
            rb_ps = p["ps_d"].tile([128, TB], F32, name="rb_ps", tag="dps")
            nc.tensor.matmul(
                rb_ps[:], ones_row[:], rhs=rec_sb[:], start=True, stop=True
            )
            rb_sb = p["misc"].tile([128, TB], F32, name="rb_sb")
            nc.scalar.copy(rb_sb[:], rb_ps[:])
            nc.vector.tensor_mul(o_sb[:, ts], o_ps[:], rb_sb[:])

        # software pipeline: S(j+1) emitted before o(j) so the exp latency
        # of e(j) hides under S(j+1)'s matmul + interleaved filler
        items = [lambda: sf(0)]
        for j in range(1, njs):
            items.append(lambda j=j: (sf(j), of(j - 1)) and None)
        items.append(lambda: of(njs - 1))
        items.append(fin)
        return items

    TBP = 256  # proj output-column chunk (8 chunks; small wt tiles)

    def make_proj(b):
        wt_holder = {}

        def load_wt(ob):
            def f():
                wt = p["wp"].tile([128, KT, TBP], BF16, name="wt")
                # host pre-packed [ob, p, kt, o]: contiguous 8KB/partition
                nc.sync.dma_start(out=wt[:], in_=io["w_proj_bf"][ob])
                wt_holder[ob] = wt
            return f

        def pchunk(ob, h):
            def f():
                wt = wt_holder[ob]
                os_ = slice(ob * TBP, (ob + 1) * TBP)
                y_ps = p["ps_mm"].tile([128, TBP], F32, name="y_ps", tag="mmps")
                o_sb = out_sb[(b, h)]
                for kt in range(KT):
                    lhsT = o_sb.rearrange("d (t2 g) -> d g t2", g=16)[:, kt]
                    nc.tensor.matmul(
                        y_ps[:],
                        lhsT,
                        rhs=wt[:, kt],
                        start=(kt == 0),
                        stop=(kt == KT - 1),
                    )
                y_sb = p["misc"].tile([128, TBP], F32, name="y_sb")
                nc.scalar.copy(y_sb[:], y_ps[:])
                # b=1 stores go out on the scalar queue: the sync queue is
                # saturated with wt reloads during the proj(b1) phase
                eng = nc.scalar if b == 1 else nc.sync
                eng.dma_start(out=io["y"][b, h, :, os_], in_=y_sb[:])
            return f

        return load_wt, pchunk

    # ---- merged pipeline ----
    filler = deque()

    def drain(n):
        for _ in range(n):
            if not filler:
                return
            filler.popleft()()

    for b in (0, 1):
        groups = make_qkv(b)
        for tb in range(NTB):
            for c in groups[tb]:
                c()
                drain(2)
            for h in range(HL):
                filler.extend(attn_block_items(b, h, tb))

    lw0, pc0 = make_proj(0)
    lw1, pc1 = make_proj(1)
    NOB = C // TBP
    c_items = []
    for ob in range(NOB - 1):
        c_items += [lw0(ob), pc0(ob, 0), pc0(ob, 1)]
    # prefetch first two b=1 wt chunks during phase C (wp bufs=4)
    c_items += [lw0(NOB - 1), lw1(0), lw1(1), pc0(NOB - 1, 0), lw1(2),
                pc0(NOB - 1, 1), lw1(3), lw1(4), lw1(5)]
    rem = list(filler)
    filler.clear()
    # Interleave leftover attention only with the first ~60% of proj(b0):
    # the proj-only tail covers the last finalize chains' latency so
    # proj(b1) never waits on a just-normalized o_sb.
    nhead = max(1, (len(c_items) * 3) // 5)
    _interleave(iter(c_items[:nhead]), iter(rem),
                ratio=max(1, len(rem)) / nhead)
    for f in c_items[nhead:]:
        f()
    d_items = []
    for ob in range(NOB):
        d_items += [pc1(ob, 0), pc1(ob, 1)]
        if ob + 6 < NOB:
            # reload into the buffer pc1(ob,*) just released (wp bufs=6)
            d_items.append(lw1(ob + 6))
    for f in d_items:
        f()


def _build(reps=1):
    from concourse import bacc
    import concourse.mybir as mybir
    import concourse.tile as tile

    F32 = mybir.dt.float32
    BF16 = mybir.dt.bfloat16

    nc = bacc.Bacc(None, target_bir_lowering=False)
    io = {
        "x_bf": nc.dram_tensor("x_bf", [B, C, T], BF16, kind="ExternalInput"),
        "w_qkv_bf": nc.dram_tensor("w_qkv_bf", [C, 6, 128], BF16,
                                   kind="ExternalInput"),
        "w_proj_bf": nc.dram_tensor("w_proj_bf", [C // 256, 128, C // 128, 256],
                                    BF16, kind="ExternalInput"),
        "sin_t": nc.dram_tensor("sin_t", [64, T], BF16, kind="ExternalInput"),
        "cos_t": nc.dram_tensor("cos_t", [64, T], BF16, kind="ExternalInput"),
        "tri": nc.dram_tensor("tri", [128, 128], BF16, kind="ExternalInput"),
        "y": nc.dram_tensor("y", [B, HL, 128, C], F32, kind="ExternalOutput"),
    }
    with tile.TileContext(nc) as tc, ExitStack() as ctx:
        pools = {
            "const": ctx.enter_context(tc.tile_pool(name="const", bufs=1)),
            "ps_mm": ctx.enter_context(
                tc.tile_pool(name="ps_mm", bufs=2, space="PSUM")),
            "ps_s": ctx.enter_context(
                tc.tile_pool(name="ps_s", bufs=3, space="PSUM")),
            "ps_o": ctx.enter_context(
                tc.tile_pool(name="ps_o", bufs=2, space="PSUM")),
            "ps_d": ctx.enter_context(
                tc.tile_pool(name="ps_d", bufs=1, space="PSUM")),
            "xp": ctx.enter_context(tc.tile_pool(name="xp", bufs=2)),
            "qkvp": ctx.enter_context(tc.tile_pool(name="qkvp", bufs=2)),
            "ep": ctx.enter_context(tc.tile_pool(name="ep", bufs=4)),
            "accp": ctx.enter_context(tc.tile_pool(name="accp", bufs=2)),
            "outp": ctx.enter_context(tc.tile_pool(name="outp", bufs=1)),
            "wp": ctx.enter_context(tc.tile_pool(name="wp", bufs=6)),
            "misc": ctx.enter_context(tc.tile_pool(name="misc", bufs=2)),
        }
        for _ in range(reps):
            _emit(nc, io, pools, mybir)
    nc.compile()
    return nc


def _make_executor(nc):
    import jax
    from jax.sharding import Mesh, NamedSharding, PartitionSpec
    from jax.experimental.shard_map import shard_map
    import concourse.mybir as mybir
    from concourse.bass2jax import (
        _bass_exec_p,
        install_neuronx_cc_hook,
        partition_id_tensor,
    )

    install_neuronx_cc_hook()
    partition_name = (
        nc.partition_id_tensor.name if nc.partition_id_tensor else None
    )
    in_names, out_names, out_avals, zero_outs = [], [], [], []
    for alloc in nc.m.functions[0].allocations:
        if not isinstance(alloc, mybir.MemoryLocationSet):
            continue
        name = alloc.memorylocations[0].name
        if alloc.kind == "ExternalInput":
            if name != partition_name:
                in_names.append(name)
        elif alloc.kind == "ExternalOutput":
            shape = tuple(alloc.tensor_shape)
            dtype = mybir.dt.np(alloc.dtype)
            out_names.append(name)
            out_avals.append(jax.core.ShapedArray(shape, dtype))
            zero_outs.append(np.zeros(shape, dtype))
    n_params = len(in_names)
    n_outs = len(out_avals)
    in_names.extend(out_names)
    if partition_name is not None:
        in_names.append(partition_name)
    donate = tuple(range(n_params, n_params + n_outs))

    def _body(*args):
        operands = list(args)
        if partition_name is not None:
            operands.append(partition_id_tensor())
        return tuple(
            _bass_exec_p.bind(
                *operands,
                out_avals=tuple(out_avals),
                in_names=tuple(in_names),
                out_names=tuple(out_names),
                lowering_input_output_aliases=(),
                sim_require_finite=True,
                sim_require_nnan=True,
                nc=nc,
            )
        )

    devices = jax.devices()[:N_CORES]
    assert len(devices) == N_CORES, f"need {N_CORES} cores, got {len(devices)}"
    mesh = Mesh(np.asarray(devices), ("core",))
    in_specs = (PartitionSpec("core"),) * (n_params + n_outs)
    out_specs = (PartitionSpec("core"),) * len(out_names)
    sharded = jax.jit(
        shard_map(_body, mesh=mesh, in_specs=in_specs, out_specs=out_specs,
                  check_rep=False),
        donate_argnums=donate,
        keep_unused=True,
    )

    def run(in_maps):
        per_core = [
            [np.asarray(m[name]) for name in in_names[:n_params]]
            for m in in_maps
        ]
        concat_in = [
            np.concatenate([per_core[c][i] for c in range(N_CORES)], axis=0)
            for i in range(n_params)
        ]
        concat_zeros = [
            np.zeros((N_CORES * z.shape[0], *z.shape[1:]), z.dtype)
            for z in zero_outs
        ]
        out_arrs = sharded(*concat_in, *concat_zeros)
        jax.block_until_ready(out_arrs)
        return [
            {
                name: np.asarray(out_arrs[i]).reshape(
                    N_CORES, *out_avals[i].shape
                )[c]
                for i, name in enumerate(out_names)
            }
            for c in range(N_CORES)
        ]

    return run


def _host_prep(x, w_qkv, w_proj):
    import ml_dtypes

    bf = ml_dtypes.bfloat16
    x = np.asarray(x, dtype=np.float32)
    w_qkv = np.asarray(w_qkv, dtype=np.float32)
    w_proj = np.asarray(w_proj, dtype=np.float32)

    x_bf = np.ascontiguousarray(x.transpose(0, 2, 1)).astype(bf)  # [B, C, T]
    # [c_in, o] -> [ob, p, kt, o_local]: the proj wt tile layout, so each
    # chunk DMA reads contiguous 8KB per partition
    wp_t = w_proj.T.reshape(16, 128, 8, 256)
    w_proj_bf = np.ascontiguousarray(wp_t.transpose(2, 1, 0, 3)).astype(bf)

    pos = np.arange(T, dtype=np.float32)[:, None]
    inv = np.exp(
        np.arange(0, D, 2, dtype=np.float32) * np.float32(-math.log(10000.0) / D)
    )
    ang = pos * inv
    sin_t = np.ascontiguousarray(np.sin(ang).astype(np.float32).T).astype(bf)
    cos_t = np.ascontiguousarray(np.cos(ang).astype(np.float32).T).astype(bf)
    tri = np.triu(np.ones((128, 128), dtype=np.float32)).astype(bf)

    in_maps = []
    for c in range(N_CORES):
        h0, h1 = 2 * c, 2 * c + 1
        blocks = []
        for base in (0, C, 2 * C):  # q, k, v feature rows
            for h in (h0, h1):
                blocks.append(w_qkv[base + h * D : base + (h + 1) * D, :])
        w_slab = np.stack(blocks, 0)  # [6, 128, C]
        w_t = np.ascontiguousarray(w_slab.transpose(2, 0, 1)).astype(bf)
        in_maps.append(
            {
                "x_bf": x_bf,
                "w_qkv_bf": w_t,
                "w_proj_bf": w_proj_bf,
                "sin_t": sin_t,
                "cos_t": cos_t,
                "tri": tri,
            }
        )
    return in_maps


def kernel(x, w_qkv, w_proj):
    """Full inputs in, full output out. Shards over 8 NeuronCores inside."""
    if "run" not in _CACHE:
        nc = _build()
        _CACHE["run"] = _make_executor(nc)
    run = _CACHE["run"]
    in_maps = _host_prep(x, w_qkv, w_proj)
    outs = run(in_maps)
    y = np.empty((B, T, C), dtype=np.float32)
    for c in range(N_CORES):
        for hl in range(HL):
            h = 2 * c + hl
            y[:, h * 128 : (h + 1) * 128, :] = outs[c]["y"][:, hl]
    return y

